# revision 1
# baseline (speedup 1.0000x reference)
"""Trainium2 Bass kernel for nn_Decoder (GNN edge MLP).

  out[e] = W2 @ relu(W1 @ [z[row_e]; z[col_e]] + b1) + b2

Algorithm (per core, edges sharded 8 ways):
  1. Device prologue: TAB[t] = [ |W2|*(z[t]@W1a.T + b1) ; |W2|*(z[t]@W1b.T) ]
     (fp32 [100000, 128] table in DRAM, channels permuted so W2>0 first),
     computed with PE matmuls from host-prepped zT1 = [z.T; 1], Wab.
  2. Main loop: edges bucketed by (row//25000, col//25000) on host (16
     buckets, shared tile plan across cores). Per tile of <=8064 edges:
     dma_gather A-half rows by row idx, B-half rows by col idx (int16
     window-local indices), add, relu, then signed segment-reduce:
     out = sum(h[:kpos]) - sum(h[kpos:]) + b2.
  3. Host unshards/unpermutes the [128, NJ] per-core outputs.
"""
import sys
sys.path.insert(0, "/opt/trn_rl_repo")
import time
import numpy as np

import concourse.bacc as bacc
import concourse.bass as bass
import concourse.tile as tile
from concourse import mybir
from concourse._compat import cdiv

N_NODES = 100000
NHID = 64
E_TOTAL = 3200000
N_CORES = 8
W = 25000          # gather index window (< 32768 for int16)
NW = 4
NBKT = NW * NW
T_MAX = 8064       # idxs per dma_gather (8064/16+2=506 descs/ring < 1022)
ZCHUNK = 512       # prologue tokens per chunk


# ---------------------------------------------------------------- host prep

def _plan_and_pack(row, col):
    E = row.shape[0]
    Ec = E // N_CORES
    rows = row.reshape(N_CORES, Ec).astype(np.int64)
    cols = col.reshape(N_CORES, Ec).astype(np.int64)

    bkt = (rows // W) * NW + (cols // W)
    order = np.argsort(bkt, axis=1, kind="stable")
    bkt_sorted = np.take_along_axis(bkt, order, axis=1)
    counts = np.stack([np.bincount(bkt_sorted[c], minlength=NBKT)
                       for c in range(N_CORES)])
    padded = ((counts.max(axis=0) + 127) // 128) * 128
    padded = np.maximum(padded, 128)

    tiles = []
    idx_off = out_off = 0
    for b in range(NBKT):
        rem = int(padded[b])
        while rem > 0:
            Tt = min(rem, T_MAX)
            tiles.append((b, Tt, idx_off, out_off))
            idx_off += Tt // 16
            out_off += Tt // 128
            rem -= Tt
    NC, NJ = idx_off, out_off

    idxr = np.zeros((N_CORES, 16, NC), np.int16)
    idxc = np.zeros((N_CORES, 16, NC), np.int16)
    up = np.empty((N_CORES, Ec), np.int64)
    uc = np.empty((N_CORES, Ec), np.int64)

    slot_maps = {}
    for b in range(NBKT):
        P = int(padded[b])
        perpart = np.empty(P, np.int64)
        percol = np.empty(P, np.int64)
        perflat = np.empty(P, np.int64)
        pos = 0
        for (bb, Tt, ioff, ooff) in tiles:
            if bb != b:
                continue
            sl = np.arange(Tt)
            perpart[pos:pos + Tt] = sl % 128
            percol[pos:pos + Tt] = ooff + sl // 128
            perflat[pos:pos + Tt] = ioff * 16 + sl
            pos += Tt
        slot_maps[b] = (perpart, percol, perflat)

    bucket_starts = np.zeros((N_CORES, NBKT + 1), np.int64)
    bucket_starts[:, 1:] = np.cumsum(counts, axis=1)
    for c in range(N_CORES):
        r_s = rows[c][order[c]]
        c_s = cols[c][order[c]]
        for b in range(NBKT):
            s0, s1 = int(bucket_starts[c, b]), int(bucket_starts[c, b + 1])
            n = s1 - s0
            if n == 0:
                continue
            br, bc = divmod(b, NW)
            perpart, percol, perflat = slot_maps[b]
            ks = perflat[:n]
            idxr[c, ks % 16, ks // 16] = (r_s[s0:s1] - br * W).astype(np.int16)
            idxc[c, ks % 16, ks // 16] = (c_s[s0:s1] - bc * W).astype(np.int16)
            up[c, order[c][s0:s1]] = perpart[:n]
            uc[c, order[c][s0:s1]] = percol[:n]
    idxr = np.tile(idxr, (1, 8, 1))
    idxc = np.tile(idxc, (1, 8, 1))
    return idxr, idxc, tiles, NC, NJ, up, uc


# ------------------------------------------------------------- bass program

def _build_program(tiles, NC, NJ, kpos, b2val, repeats=1):
    nc = bacc.Bacc("TRN2", target_bir_lowering=False, debug=False,
                   num_devices=N_CORES)
    f32 = mybir.dt.float32
    zT1_d = nc.dram_tensor("zT1", [NHID + 1, N_NODES], f32, kind="ExternalInput")
    Wab_d = nc.dram_tensor("Wab", [NHID + 1, 2 * NHID], f32, kind="ExternalInput")
    idxr_d = nc.dram_tensor("idxr", [128, NC], mybir.dt.int16, kind="ExternalInput")
    idxc_d = nc.dram_tensor("idxc", [128, NC], mybir.dt.int16, kind="ExternalInput")
    out_d = nc.dram_tensor("out", [128, NJ], f32, kind="ExternalOutput")
    tab_d = nc.dram_tensor("tab", [N_NODES, 2 * NHID], f32, kind="Internal")

    nzchunks = cdiv(N_NODES, ZCHUNK)

    with tile.TileContext(nc) as tc:
        with (
            tc.tile_pool(name="w", bufs=1) as wpool,
            tc.tile_pool(name="zt", bufs=3) as zpool,
            tc.tile_pool(name="ps", bufs=4, space="PSUM") as pspool,
            tc.tile_pool(name="st", bufs=3) as stpool,
            tc.tile_pool(name="g", bufs=3) as gpool,
            tc.tile_pool(name="ix", bufs=3) as ixpool,
            tc.tile_pool(name="r", bufs=3) as rpool,
            tc.tile_pool(name="oa", bufs=1) as oapool,
        ):
            # ---------------- prologue: build TAB in DRAM
            wab_t = wpool.tile([NHID + 1, 2 * NHID], f32)
            nc.sync.dma_start(wab_t[:], Wab_d.ap()[:])
            for ci in range(nzchunks):
                t0 = ci * ZCHUNK
                t1 = min(t0 + ZCHUNK, N_NODES)
                ntok = t1 - t0
                zt = zpool.tile([NHID + 1, ZCHUNK], f32, tag="zt")
                nc.sync.dma_start(zt[:, :ntok], zT1_d.ap()[:, t0:t1])
                stage = stpool.tile([128, ZCHUNK // 128, 2 * NHID], f32, tag="stage")
                for m in range(cdiv(ntok, 128)):
                    m0 = m * 128
                    mt = min(128, ntok - m0)
                    ps = pspool.tile([128, 2 * NHID], f32, tag="ps")
                    nc.tensor.matmul(
                        out=ps[:mt, :], lhsT=zt[:, m0:m0 + mt], rhs=wab_t[:],
                        start=True, stop=True,
                    )
                    nc.scalar.activation(
                        out=stage[:mt, m, :], in_=ps[:mt, :],
                        func=mybir.ActivationFunctionType.Copy,
                    )
                # write chunk rows to TAB
                for m in range(cdiv(ntok, 128)):
                    m0 = m * 128
                    mt = min(128, ntok - m0)
                    nc.sync.dma_start(
                        tab_d.ap()[t0 + m0:t0 + m0 + mt, :],
                        stage[:mt, m, :],
                    )

            # ---------------- main loop
            outacc = oapool.tile([128, NJ], f32)
            for rep in range(repeats):
                for (b, Tt, ioff, ooff) in tiles:
                    br, bc = divmod(b, NW)
                    J = Tt // 128
                    ir = ixpool.tile([128, T_MAX // 16], mybir.dt.int16, tag="ir")
                    ic = ixpool.tile([128, T_MAX // 16], mybir.dt.int16, tag="ic")
                    nc.sync.dma_start(ir[:, :Tt // 16], idxr_d.ap()[:, ioff:ioff + Tt // 16])
                    nc.sync.dma_start(ic[:, :Tt // 16], idxc_d.ap()[:, ioff:ioff + Tt // 16])
                    gr = gpool.tile([128, T_MAX // 128, NHID], f32, tag="gr")
                    gc = gpool.tile([128, T_MAX // 128, NHID], f32, tag="gc")
                    in_r = tab_d.ap()[br * W:(br + 1) * W, :NHID]
                    in_c = tab_d.ap()[bc * W:(bc + 1) * W, NHID:]
                    nc.gpsimd.dma_gather(
                        gr[:, :J, :], in_r, ir[:, :Tt // 16], Tt, Tt, NHID,
                        elem_step=2 * NHID, single_packet=False,
                    )
                    nc.gpsimd.dma_gather(
                        gc[:, :J, :], in_c, ic[:, :Tt // 16], Tt, Tt, NHID,
                        elem_step=2 * NHID, single_packet=False,
                    )
                    nc.vector.tensor_tensor(
                        out=gr[:, :J, :], in0=gr[:, :J, :], in1=gc[:, :J, :],
                        op=mybir.AluOpType.add,
                    )
                    nc.scalar.activation(
                        out=gr[:, :J, :], in_=gr[:, :J, :],
                        func=mybir.ActivationFunctionType.Relu,
                    )
                    rp = rpool.tile([128, T_MAX // 128], f32, tag="rp")
                    rn = rpool.tile([128, T_MAX // 128], f32, tag="rn")
                    nc.vector.tensor_reduce(
                        out=rp[:, :J], in_=gr[:, :J, :kpos],
                        axis=mybir.AxisListType.X, op=mybir.AluOpType.add,
                    )
                    nc.vector.tensor_reduce(
                        out=rn[:, :J], in_=gr[:, :J, kpos:],
                        axis=mybir.AxisListType.X, op=mybir.AluOpType.add,
                    )
                    nc.vector.tensor_tensor(
                        out=outacc[:, ooff:ooff + J], in0=rp[:, :J], in1=rn[:, :J],
                        op=mybir.AluOpType.subtract,
                    )
            nc.vector.tensor_scalar_add(
                out=outacc[:], in0=outacc[:], scalar1=float(b2val),
            )
            nc.sync.dma_start(out_d.ap()[:], outacc[:])
    nc.compile()
    return nc


# ------------------------------------------------------------------ runner

class _SpmdRunner:
    def __init__(self, nc, n_cores):
        import jax
        from jax.sharding import Mesh, PartitionSpec
        from jax.experimental.shard_map import shard_map
        from concourse.bass2jax import (
            install_neuronx_cc_hook, _bass_exec_p, partition_id_tensor,
        )
        install_neuronx_cc_hook()
        self.jax = jax
        self.nc = nc
        self.n_cores = n_cores
        partition_name = nc.partition_id_tensor.name if nc.partition_id_tensor else None
        in_names, out_names, out_avals, zero_outs = [], [], [], []
        for alloc in nc.m.functions[0].allocations:
            if not isinstance(alloc, mybir.MemoryLocationSet):
                continue
            name = alloc.memorylocations[0].name
            if alloc.kind == "ExternalInput":
                if name != partition_name:
                    in_names.append(name)
            elif alloc.kind == "ExternalOutput":
                out_names.append(name)
                shape = tuple(alloc.tensor_shape)
                dtype = mybir.dt.np(alloc.dtype)
                out_avals.append(jax.core.ShapedArray(shape, dtype))
                zero_outs.append(np.zeros(shape, dtype))
        self.in_names, self.out_names = in_names, out_names
        self.out_avals, self.zero_outs = out_avals, zero_outs
        n_params, n_outs = len(in_names), len(out_avals)
        all_in_names = list(in_names) + list(out_names)
        if partition_name is not None:
            all_in_names.append(partition_name)
        donate = tuple(range(n_params, n_params + n_outs))

        def _body(*args):
            operands = list(args)
            if partition_name is not None:
                operands.append(partition_id_tensor())
            outs = _bass_exec_p.bind(
                *operands,
                out_avals=tuple(out_avals),
                in_names=tuple(all_in_names),
                out_names=tuple(out_names),
                lowering_input_output_aliases=(),
                sim_require_finite=True,
                sim_require_nnan=True,
                nc=nc,
            )
            return tuple(outs)

        devices = jax.devices()[:n_cores]
        mesh = Mesh(np.asarray(devices), ("core",))
        in_specs = (PartitionSpec("core"),) * (n_params + n_outs)
        out_specs = (PartitionSpec("core"),) * len(out_names)
        self._fn = jax.jit(
            shard_map(_body, mesh=mesh, in_specs=in_specs,
                      out_specs=out_specs, check_rep=False),
            donate_argnums=donate, keep_unused=True,
        )

    def run(self, in_maps):
        jax = self.jax
        concat = [np.concatenate([np.asarray(m[n]) for m in in_maps], axis=0)
                  for n in self.in_names]
        zeros = [np.zeros((self.n_cores * z.shape[0], *z.shape[1:]), z.dtype)
                 for z in self.zero_outs]
        out_arrs = self._fn(*concat, *zeros)
        jax.block_until_ready(out_arrs)
        return [
            {n: np.asarray(out_arrs[i]).reshape(self.n_cores, *self.out_avals[i].shape)[c]
             for i, n in enumerate(self.out_names)}
            for c in range(self.n_cores)
        ]


# ------------------------------------------------------------------ kernel

_CACHE = {}


def _prepare(z, row, col, W1, b1, W2, b2, repeats=1):
    w2 = np.asarray(W2, np.float32).reshape(-1)
    perm = np.argsort(w2 <= 0, kind="stable")
    kpos = int((w2 > 0).sum())
    aw2 = np.abs(w2)[perm]

    W1a = np.asarray(W1, np.float32)[:, :NHID]
    W1b = np.asarray(W1, np.float32)[:, NHID:]
    # Wab: [65, 128]; columns 0:64 -> |W2|-scaled permuted A-channels (+b1 row),
    # 64:128 -> B-channels
    Wab = np.zeros((NHID + 1, 2 * NHID), np.float32)
    Wab[:NHID, :NHID] = W1a.T[:, perm] * aw2
    Wab[NHID, :NHID] = (np.asarray(b1, np.float32)[perm] * aw2)
    Wab[:NHID, NHID:] = W1b.T[:, perm] * aw2

    zT1 = np.empty((NHID + 1, N_NODES), np.float32)
    zT1[:NHID] = np.asarray(z, np.float32).T
    zT1[NHID] = 1.0

    idxr, idxc, tiles, NC, NJ, up, uc = _plan_and_pack(
        np.asarray(row), np.asarray(col))

    key = (tuple(t[1] for t in tiles), NC, NJ, kpos, float(np.asarray(b2).reshape(-1)[0]), repeats)
    if key not in _CACHE:
        nc = _build_program(tiles, NC, NJ, kpos, float(np.asarray(b2).reshape(-1)[0]), repeats)
        runner = _SpmdRunner(nc, N_CORES)
        _CACHE[key] = runner
    runner = _CACHE[key]

    in_maps = []
    for c in range(N_CORES):
        in_maps.append({
            "zT1": zT1, "Wab": Wab,
            "idxr": idxr[c], "idxc": idxc[c],
        })
    return runner, in_maps, up, uc


def kernel(z, row, col, W1, b1, W2, b2):
    E = np.asarray(row).shape[0]
    runner, in_maps, up, uc = _prepare(z, row, col, W1, b1, W2, b2)
    results = runner.run(in_maps)
    Ec = E // N_CORES
    out = np.empty(E, np.float32)
    for c in range(N_CORES):
        out[c * Ec:(c + 1) * Ec] = results[c]["out"][up[c], uc[c]]
    return out



# revision 3
# speedup vs baseline: 108.4365x; 108.4365x over previous
"""Trainium2 Bass kernel for nn_Decoder (GNN edge MLP).

  out[e] = W2 @ relu(W1 @ [z[row_e]; z[col_e]] + b1) + b2

Algorithm (per core, edges sharded 8 ways):
  1. Device prologue: TAB[t] = [ |W2|*(z[t]@W1a.T + b1) ; |W2|*(z[t]@W1b.T) ]
     (fp32 [100000, 128] table in DRAM, channels permuted so W2>0 first),
     computed with PE matmuls from host-prepped zT1 = [z.T; 1], Wab.
  2. Main loop: edges bucketed by (row//25000, col//25000) on host (16
     buckets, shared tile plan across cores). Per tile of <=8064 edges:
     dma_gather A-half rows by row idx, B-half rows by col idx (int16
     window-local indices), add, relu, then signed segment-reduce:
     out = sum(h[:kpos]) - sum(h[kpos:]) + b2.
  3. Host unshards/unpermutes the [128, NJ] per-core outputs.
"""
import sys
sys.path.insert(0, "/opt/trn_rl_repo")
import time
import numpy as np

import concourse.bacc as bacc
import concourse.bass as bass
import concourse.tile as tile
from concourse import mybir
from concourse._compat import cdiv

N_NODES = 100000
NHID = 64
E_TOTAL = 3200000
N_CORES = 8
W = 25000          # gather index window (< 32768 for int16)
NW = 4
NBKT = NW * NW
T_MAX = 8064       # idxs per dma_gather (8064/16+2=506 descs/ring < 1022)
ZCHUNK = 512       # prologue tokens per chunk


# ---------------------------------------------------------------- host prep

def _plan_and_pack(row, col):
    E = row.shape[0]
    Ec = E // N_CORES
    rows = row.reshape(N_CORES, Ec).astype(np.int64)
    cols = col.reshape(N_CORES, Ec).astype(np.int64)

    bkt = (rows // W) * NW + (cols // W)
    order = np.argsort(bkt, axis=1, kind="stable")
    bkt_sorted = np.take_along_axis(bkt, order, axis=1)
    counts = np.stack([np.bincount(bkt_sorted[c], minlength=NBKT)
                       for c in range(N_CORES)])
    padded = ((counts.max(axis=0) + 127) // 128) * 128
    padded = np.maximum(padded, 128)

    tiles = []
    idx_off = out_off = 0
    for b in range(NBKT):
        rem = int(padded[b])
        while rem > 0:
            Tt = min(rem, T_MAX)
            tiles.append((b, Tt, idx_off, out_off))
            idx_off += Tt // 16
            out_off += Tt // 128
            rem -= Tt
    NC, NJ = idx_off, out_off

    idxr = np.zeros((N_CORES, 16, NC), np.int16)
    idxc = np.zeros((N_CORES, 16, NC), np.int16)
    up = np.empty((N_CORES, Ec), np.int64)
    uc = np.empty((N_CORES, Ec), np.int64)

    slot_maps = {}
    for b in range(NBKT):
        P = int(padded[b])
        perpart = np.empty(P, np.int64)
        percol = np.empty(P, np.int64)
        perflat = np.empty(P, np.int64)
        pos = 0
        for (bb, Tt, ioff, ooff) in tiles:
            if bb != b:
                continue
            sl = np.arange(Tt)
            perpart[pos:pos + Tt] = sl % 128
            percol[pos:pos + Tt] = ooff + sl // 128
            perflat[pos:pos + Tt] = ioff * 16 + sl
            pos += Tt
        slot_maps[b] = (perpart, percol, perflat)

    bucket_starts = np.zeros((N_CORES, NBKT + 1), np.int64)
    bucket_starts[:, 1:] = np.cumsum(counts, axis=1)
    for c in range(N_CORES):
        r_s = rows[c][order[c]]
        c_s = cols[c][order[c]]
        for b in range(NBKT):
            s0, s1 = int(bucket_starts[c, b]), int(bucket_starts[c, b + 1])
            n = s1 - s0
            if n == 0:
                continue
            br, bc = divmod(b, NW)
            perpart, percol, perflat = slot_maps[b]
            ks = perflat[:n]
            idxr[c, ks % 16, ks // 16] = (r_s[s0:s1] - br * W).astype(np.int16)
            idxc[c, ks % 16, ks // 16] = (c_s[s0:s1] - bc * W).astype(np.int16)
            up[c, order[c][s0:s1]] = perpart[:n]
            uc[c, order[c][s0:s1]] = percol[:n]
    idxr = np.tile(idxr, (1, 8, 1))
    idxc = np.tile(idxc, (1, 8, 1))
    return idxr, idxc, tiles, NC, NJ, up, uc


# ------------------------------------------------------------- bass program

def _build_program(tiles, NC, NJ, kpos, b2val, repeats=1):
    nc = bacc.Bacc("TRN2", target_bir_lowering=False, debug=False,
                   num_devices=N_CORES)
    f32 = mybir.dt.float32
    zT1_d = nc.dram_tensor("zT1", [NHID + 1, N_NODES], f32, kind="ExternalInput")
    Wab_d = nc.dram_tensor("Wab", [NHID + 1, 2 * NHID], f32, kind="ExternalInput")
    idxr_d = nc.dram_tensor("idxr", [128, NC], mybir.dt.int16, kind="ExternalInput")
    idxc_d = nc.dram_tensor("idxc", [128, NC], mybir.dt.int16, kind="ExternalInput")
    out_d = nc.dram_tensor("out", [128, NJ], f32, kind="ExternalOutput")
    tab_d = nc.dram_tensor("tab", [N_NODES, 2 * NHID], f32, kind="Internal")

    nzchunks = cdiv(N_NODES, ZCHUNK)

    with tile.TileContext(nc) as tc:
        with (
            tc.tile_pool(name="w", bufs=1) as wpool,
            tc.tile_pool(name="zt", bufs=3) as zpool,
            tc.tile_pool(name="ps", bufs=4, space="PSUM") as pspool,
            tc.tile_pool(name="st", bufs=3) as stpool,
            tc.tile_pool(name="g", bufs=3) as gpool,
            tc.tile_pool(name="ix", bufs=3) as ixpool,
            tc.tile_pool(name="r", bufs=3) as rpool,
            tc.tile_pool(name="oa", bufs=1) as oapool,
        ):
            # ---------------- prologue: build TAB in DRAM
            wab_t = wpool.tile([NHID + 1, 2 * NHID], f32)
            nc.sync.dma_start(wab_t[:], Wab_d.ap()[:])
            for ci in range(nzchunks):
                t0 = ci * ZCHUNK
                t1 = min(t0 + ZCHUNK, N_NODES)
                ntok = t1 - t0
                zt = zpool.tile([NHID + 1, ZCHUNK], f32, tag="zt")
                nc.sync.dma_start(zt[:, :ntok], zT1_d.ap()[:, t0:t1])
                stage = stpool.tile([128, ZCHUNK // 128, 2 * NHID], f32, tag="stage")
                for m in range(cdiv(ntok, 128)):
                    m0 = m * 128
                    mt = min(128, ntok - m0)
                    ps = pspool.tile([128, 2 * NHID], f32, tag="ps")
                    nc.tensor.matmul(
                        out=ps[:mt, :], lhsT=zt[:, m0:m0 + mt], rhs=wab_t[:],
                        start=True, stop=True,
                    )
                    nc.scalar.activation(
                        out=stage[:mt, m, :], in_=ps[:mt, :],
                        func=mybir.ActivationFunctionType.Copy,
                    )
                # write chunk rows to TAB
                for m in range(cdiv(ntok, 128)):
                    m0 = m * 128
                    mt = min(128, ntok - m0)
                    nc.sync.dma_start(
                        tab_d.ap()[t0 + m0:t0 + m0 + mt, :],
                        stage[:mt, m, :],
                    )

            # ---------------- main loop
            outacc = oapool.tile([128, NJ], f32)
            for rep in range(repeats):
                for (b, Tt, ioff, ooff) in tiles:
                    br, bc = divmod(b, NW)
                    J = Tt // 128
                    ir = ixpool.tile([128, T_MAX // 16], mybir.dt.int16, tag="ir")
                    ic = ixpool.tile([128, T_MAX // 16], mybir.dt.int16, tag="ic")
                    nc.sync.dma_start(ir[:, :Tt // 16], idxr_d.ap()[:, ioff:ioff + Tt // 16])
                    nc.sync.dma_start(ic[:, :Tt // 16], idxc_d.ap()[:, ioff:ioff + Tt // 16])
                    gr = gpool.tile([128, T_MAX // 128, NHID], f32, tag="gr")
                    gc = gpool.tile([128, T_MAX // 128, NHID], f32, tag="gc")
                    in_r = tab_d.ap()[br * W:(br + 1) * W, :NHID]
                    in_c = tab_d.ap()[bc * W:(bc + 1) * W, NHID:]
                    nc.gpsimd.dma_gather(
                        gr[:, :J, :], in_r, ir[:, :Tt // 16], Tt, Tt, NHID,
                        elem_step=2 * NHID, single_packet=False,
                    )
                    nc.gpsimd.dma_gather(
                        gc[:, :J, :], in_c, ic[:, :Tt // 16], Tt, Tt, NHID,
                        elem_step=2 * NHID, single_packet=False,
                    )
                    nc.vector.tensor_tensor(
                        out=gr[:, :J, :], in0=gr[:, :J, :], in1=gc[:, :J, :],
                        op=mybir.AluOpType.add,
                    )
                    nc.scalar.activation(
                        out=gr[:, :J, :], in_=gr[:, :J, :],
                        func=mybir.ActivationFunctionType.Relu,
                    )
                    rp = rpool.tile([128, T_MAX // 128], f32, tag="rp")
                    rn = rpool.tile([128, T_MAX // 128], f32, tag="rn")
                    nc.vector.tensor_reduce(
                        out=rp[:, :J], in_=gr[:, :J, :kpos],
                        axis=mybir.AxisListType.X, op=mybir.AluOpType.add,
                    )
                    nc.vector.tensor_reduce(
                        out=rn[:, :J], in_=gr[:, :J, kpos:],
                        axis=mybir.AxisListType.X, op=mybir.AluOpType.add,
                    )
                    nc.vector.tensor_tensor(
                        out=outacc[:, ooff:ooff + J], in0=rp[:, :J], in1=rn[:, :J],
                        op=mybir.AluOpType.subtract,
                    )
            nc.vector.tensor_scalar_add(
                out=outacc[:], in0=outacc[:], scalar1=float(b2val),
            )
            nc.sync.dma_start(out_d.ap()[:], outacc[:])
    nc.compile()
    return nc


# ------------------------------------------------------------------ runner

class _SpmdRunner:
    def __init__(self, nc, n_cores):
        import jax
        import jax.numpy as jnp
        from jax.sharding import Mesh, PartitionSpec
        from jax.experimental.shard_map import shard_map
        from concourse.bass2jax import (
            install_neuronx_cc_hook, _bass_exec_p, partition_id_tensor,
        )
        install_neuronx_cc_hook()
        self.jax = jax
        self.nc = nc
        self.n_cores = n_cores
        partition_name = nc.partition_id_tensor.name if nc.partition_id_tensor else None
        in_names, out_names, out_avals = [], [], []
        for alloc in nc.m.functions[0].allocations:
            if not isinstance(alloc, mybir.MemoryLocationSet):
                continue
            name = alloc.memorylocations[0].name
            if alloc.kind == "ExternalInput":
                if name != partition_name:
                    in_names.append(name)
            elif alloc.kind == "ExternalOutput":
                out_names.append(name)
                shape = tuple(alloc.tensor_shape)
                dtype = mybir.dt.np(alloc.dtype)
                out_avals.append(jax.core.ShapedArray(shape, dtype))
        self.in_names, self.out_names = in_names, out_names
        self.out_avals = out_avals
        all_in_names = list(in_names) + list(out_names)
        if partition_name is not None:
            all_in_names.append(partition_name)

        def _body(*args):
            operands = list(args)
            if partition_name is not None:
                operands.append(partition_id_tensor())
            outs = _bass_exec_p.bind(
                *operands,
                out_avals=tuple(out_avals),
                in_names=tuple(all_in_names),
                out_names=tuple(out_names),
                lowering_input_output_aliases=(),
                sim_require_finite=True,
                sim_require_nnan=True,
                nc=nc,
            )
            return tuple(outs)

        devices = jax.devices()[:n_cores]
        self.mesh = Mesh(np.asarray(devices), ("core",))
        in_specs = (PartitionSpec("core"),) * (len(in_names) + len(out_names))
        out_specs = (PartitionSpec("core"),) * len(out_names)
        self._fn = jax.jit(
            shard_map(_body, mesh=self.mesh, in_specs=in_specs,
                      out_specs=out_specs, check_rep=False),
            keep_unused=True,
        )

    def device_args(self, in_maps):
        """Ship concatenated inputs + zero out-buffers to device once."""
        jax = self.jax
        from jax.sharding import NamedSharding, PartitionSpec
        sh = NamedSharding(self.mesh, PartitionSpec("core"))
        concat = [np.concatenate([np.asarray(m[n]) for m in in_maps], axis=0)
                  for n in self.in_names]
        concat += [np.zeros((self.n_cores * a.shape[0], *a.shape[1:]), a.dtype)
                   for a in self.out_avals]
        return [jax.device_put(a, sh) for a in concat]

    def run_device(self, dargs):
        out_arrs = self._fn(*dargs)
        self.jax.block_until_ready(out_arrs)
        return out_arrs

    def run(self, in_maps):
        jax = self.jax
        out_arrs = self.run_device(self.device_args(in_maps))
        return [
            {n: np.asarray(out_arrs[i]).reshape(self.n_cores, *self.out_avals[i].shape)[c]
             for i, n in enumerate(self.out_names)}
            for c in range(self.n_cores)
        ]


# ------------------------------------------------------------------ kernel

_CACHE = {}


def _prepare(z, row, col, W1, b1, W2, b2, repeats=1):
    w2 = np.asarray(W2, np.float32).reshape(-1)
    perm = np.argsort(w2 <= 0, kind="stable")
    kpos = int((w2 > 0).sum())
    aw2 = np.abs(w2)[perm]

    W1a = np.asarray(W1, np.float32)[:, :NHID]
    W1b = np.asarray(W1, np.float32)[:, NHID:]
    # Wab: [65, 128]; columns 0:64 -> |W2|-scaled permuted A-channels (+b1 row),
    # 64:128 -> B-channels
    Wab = np.zeros((NHID + 1, 2 * NHID), np.float32)
    Wab[:NHID, :NHID] = W1a.T[:, perm] * aw2
    Wab[NHID, :NHID] = (np.asarray(b1, np.float32)[perm] * aw2)
    Wab[:NHID, NHID:] = W1b.T[:, perm] * aw2

    zT1 = np.empty((NHID + 1, N_NODES), np.float32)
    zT1[:NHID] = np.asarray(z, np.float32).T
    zT1[NHID] = 1.0

    idxr, idxc, tiles, NC, NJ, up, uc = _plan_and_pack(
        np.asarray(row), np.asarray(col))

    key = (tuple(t[1] for t in tiles), NC, NJ, kpos, float(np.asarray(b2).reshape(-1)[0]), repeats)
    if key not in _CACHE:
        nc = _build_program(tiles, NC, NJ, kpos, float(np.asarray(b2).reshape(-1)[0]), repeats)
        runner = _SpmdRunner(nc, N_CORES)
        _CACHE[key] = runner
    runner = _CACHE[key]

    in_maps = []
    for c in range(N_CORES):
        in_maps.append({
            "zT1": zT1, "Wab": Wab,
            "idxr": idxr[c], "idxc": idxc[c],
        })
    return runner, in_maps, up, uc


def kernel(z, row, col, W1, b1, W2, b2):
    E = np.asarray(row).shape[0]
    runner, in_maps, up, uc = _prepare(z, row, col, W1, b1, W2, b2)
    results = runner.run(in_maps)
    Ec = E // N_CORES
    out = np.empty(E, np.float32)
    for c in range(N_CORES):
        out[c * Ec:(c + 1) * Ec] = results[c]["out"][up[c], uc[c]]
    return out

